# revision 16
# baseline (speedup 1.0000x reference)
"""Trainium2 Bass kernel for nn_DecentLayer (gnn_message_passing).

The reference gathers 16 of 24 input channels via static position matching,
then runs a 3x3 same-padded conv: [B=16, 16, 256, 256] x [32, 16, 3, 3]
-> [B, 32, 256, 256].

Strategy:
  * Data-parallel over batch: 8 cores x 2 images.
  * Per core, each image (a "phase") is split into 8 horizontal strips of 32
    output rows. Strips live on partition groups of 16 (the 16 conv input
    channels), in a zero-padded row-major layout (258 cols per row, 34 rows
    incl. halo). All 9 conv taps are then pure address offsets into this
    buffer -- no im2col copies.
  * Matmuls batch 4 strips via a block-diagonal stationary [64, 128]
    (4 x [16 ch, 32 filt]), so M=128 output partitions. Two independent
    "chains" on PE row-quadrants 0-63 / 64-127 run concurrently (distinct
    tile_positions), 4 PSUM banks each. 9 taps accumulate per PSUM bank,
    one output row (N=256) per bank.
  * Weights bf16 (FWL weight load), moving data fp32 streamed as float32r
    (1 cycle/row at N>=256). PSUM accumulates fp32; output is fp32.
"""

import numpy as np
import ml_dtypes

import concourse.bass as bass
import concourse.bacc as bacc
import concourse.mybir as mybir
import concourse.tile as tile
from concourse.bass_utils import run_bass_kernel_spmd

# Problem constants (hardcoded per the harness contract).
N_CORES = 8
B = 16
IMGS_PER_CORE = B // N_CORES  # 2
CIN = 16      # conv input channels after gather
COUT = 32     # filters
H = W = 256
SLOTS = 8     # strips per image
HS = H // SLOTS  # 32 output rows per strip
ROWS = HS + 2    # strip buffer rows incl. top/bottom halo
WP = W + 2       # padded row width
HALF = 16        # output rows per store chunk
TAPS = [(dh, dw) for dh in range(3) for dw in range(3)]

# Matmul operand mode: both operands must have matching width (compiler
# rejects mixed 32-bit / 16-bit). "f32r" = fp32 storage, float32r matmul
# (full speed at N>=256, near-fp32 accuracy). "bf16" = both bf16.
MODE = "f32r"


def _common_pairs(ms_in, ns_in, ms_x, ns_x):
    ms_in = np.asarray(ms_in)
    ns_in = np.asarray(ns_in)
    ms_x = np.asarray(ms_x)
    ns_x = np.asarray(ns_x)
    f_ids, x_ids = [], []
    for i_in in range(ms_in.shape[0]):
        hits = np.nonzero((ms_x == ms_in[i_in]) & (ns_x == ns_in[i_in]))[0]
        for i_x in hits:
            f_ids.append(i_in)
            x_ids.append(int(i_x))
    return np.asarray(f_ids), np.asarray(x_ids)


def build_program(n_img=IMGS_PER_CORE, mode=MODE):
    """Build the per-core Bass program. Returns compiled Bacc."""
    n_strips = SLOTS
    hs = HS
    rows = hs + 2
    h_img = n_strips * hs

    f32 = mybir.dt.float32
    w_dt = mybir.dt.float32r if mode == "f32r" else mybir.dt.bfloat16
    x_sb_dt = w_dt  # matmul operands must have matching width/dtype class

    nc = bacc.Bacc("TRN2", target_bir_lowering=False, debug=False)
    x_in = nc.dram_tensor("x", [n_img, CIN, h_img, W], f32, kind="ExternalInput")
    w_in = nc.dram_tensor("w", [128, 9, 128], f32, kind="ExternalInput")
    y_out = nc.dram_tensor("y", [n_img, COUT, h_img, W], f32, kind="ExternalOutput")

    # h_abs = 128*c + 32*gg + 16*m + r  ->  (c, gg, m, r)
    y_r = y_out[:].rearrange(
        "b co (c gg m r) w -> b c gg m co r w", c=2, gg=4, m=hs // HALF, r=HALF
    )

    with tile.TileContext(nc) as tc:
        with (
            tc.tile_pool(name="persist", bufs=1) as persist,
            tc.tile_pool(name="op0", bufs=2) as op0,
            tc.tile_pool(name="op1", bufs=2) as op1,
            tc.tile_pool(name="ps0", bufs=4, space="PSUM") as ps0,
            tc.tile_pool(name="ps1", bufs=4, space="PSUM") as ps1,
        ):
            out_pools = [op0, op1]
            ps_pools = [ps0, ps1]

            wt = persist.tile([128, 9, 128], w_dt, name="wt")
            nc.gpsimd.dma_start(out=wt[:], in_=w_in[:])  # casts f32 -> f32r/bf16

            xbufs = []
            for p in range(n_img):
                xb = persist.tile([128, rows * WP], x_sb_dt, name=f"xb{p}")
                xbufs.append(xb)
                xv = xb[:].rearrange("q (r c) -> q r c", c=WP)
                # zero pads: left/right columns, top halo of slot 0,
                # bottom halo of slot 7. memset can't target f32r directly
                # (ISA check), so write zero bits through a uint32 view.
                if mode == "f32r":
                    zv = lambda ap: (ap.bitcast(mybir.dt.uint32), 0)
                else:
                    zv = lambda ap: (ap, 0.0)
                nc.vector.memset(*zv(xv[:, :, 0:1]))
                nc.vector.memset(*zv(xv[:, :, WP - 1 : WP]))
                # 32-aligned partition spans (DVE base-partition constraint);
                # the non-halo half of each span is overwritten by the loads.
                nc.vector.memset(*zv(xv[0:32, 0:1, :]))
                nc.vector.memset(*zv(xv[96:128, rows - 1 : rows, :]))

            # Input loads for all phases up front (prefetch).
            for p in range(n_img):
                xv = xbufs[p][:].rearrange("q (r c) -> q r c", c=WP)
                for g in range(n_strips):
                    r0 = 1 if g == 0 else 0
                    r1 = rows - 1 if g == n_strips - 1 else rows
                    i0 = g * hs - 1 + r0  # first image row
                    dst = xv[CIN * g : CIN * (g + 1), r0:r1, 1 : W + 1]
                    src = x_in[p, :, i0 : i0 + (r1 - r0), :]
                    nc.gpsimd.dma_start(out=dst, in_=src)  # casts f32 -> f32r/bf16

            for p in range(n_img):
                xr = xbufs[p][:]
                outt = [None, None]
                for h in range(hs):
                    m, r = divmod(h, HALF)
                    for c in (0, 1):
                        ps = ps_pools[c].tile([128, W], f32, name=f"acc{c}")
                        for t, (dh, dw) in enumerate(TAPS):
                            col0 = (h + dh) * WP + dw
                            nc.tensor.matmul(
                                ps[:],
                                wt[64 * c : 64 * (c + 1), t, :],
                                xr[64 * c : 64 * (c + 1), col0 : col0 + W],
                                start=(t == 0),
                                stop=(t == 8),
                            )
                        if r == 0:
                            outt[c] = out_pools[c].tile(
                                [128, HALF * W], f32, name=f"ot{c}"
                            )
                        nc.vector.tensor_copy(outt[c][:, r * W : (r + 1) * W], ps[:])
                        if r == HALF - 1:
                            nc.sync.dma_start(out=y_r[p, c, :, m], in_=outt[c][:])

    nc.compile()
    return nc


_NC_CACHE = {}


def _get_program(mode=MODE):
    if mode not in _NC_CACHE:
        _NC_CACHE[mode] = build_program(mode=mode)
    return _NC_CACHE[mode]


def _host_prep(inputs):
    x = np.asarray(inputs["x_data"], dtype=np.float32)
    w = np.asarray(inputs["weights"], dtype=np.float32)
    f_ids, x_ids = _common_pairs(
        inputs["ms_in"], inputs["ns_in"], inputs["ms_x"], inputs["ns_x"]
    )
    assert len(f_ids) == CIN, f"expected {CIN} matched pairs, got {len(f_ids)}"
    xg = np.ascontiguousarray(x[:, x_ids])          # [B, 16, H, W]
    wg = w[:, f_ids]                                # [COUT, 16, 3, 3]

    # Stationary per tap: block-diag of 4 copies of W_t.T [16, 32] -> [64, 128],
    # replicated on partitions 64-127 for the second chain.
    w_host = np.zeros((128, 9, 128), dtype=np.float32)
    for t, (dh, dw) in enumerate(TAPS):
        lhsT = wg[:, :, dh, dw].T  # [16 ci, 32 co]
        for g in range(4):
            blk = w_host[16 * g : 16 * (g + 1), t, 32 * g : 32 * (g + 1)]
            blk[:] = lhsT
    w_host[64:128] = w_host[0:64]
    return xg, w_host


def _run(inputs, trace=False):
    xg, w_host = _host_prep(inputs)
    nc = _get_program()
    in_maps = [
        {"x": xg[IMGS_PER_CORE * k : IMGS_PER_CORE * (k + 1)], "w": w_host}
        for k in range(N_CORES)
    ]
    res = run_bass_kernel_spmd(nc, in_maps, list(range(N_CORES)), trace=trace)
    out = np.concatenate([r["y"] for r in res.results], axis=0)
    return out, res


def kernel(**inputs):
    out, _ = _run(inputs, trace=False)
    return out


# revision 17
# speedup vs baseline: 1.1368x; 1.1368x over previous
"""Trainium2 Bass kernel for nn_DecentLayer (gnn_message_passing).

The reference gathers 16 of 24 input channels via static position matching,
then runs a 3x3 same-padded conv: [B=16, 16, 256, 256] x [32, 16, 3, 3]
-> [B, 32, 256, 256].

Strategy:
  * Data-parallel over batch: 8 cores x 2 images.
  * Per core, each image (a "phase") is split into 8 horizontal strips of 32
    output rows. Strips live on partition groups of 16 (the 16 conv input
    channels), in a zero-padded row-major layout (258 cols per row, 34 rows
    incl. halo). All 9 conv taps are then pure address offsets into this
    buffer -- no im2col copies.
  * Matmuls batch 4 strips via a block-diagonal stationary [64, 128]
    (4 x [16 ch, 32 filt]), so M=128 output partitions. Two independent
    "chains" on PE row-quadrants 0-63 / 64-127 run concurrently (distinct
    tile_positions), 4 PSUM banks each. 9 taps accumulate per PSUM bank,
    one output row (N=256) per bank.
  * Weights bf16 (FWL weight load), moving data fp32 streamed as float32r
    (1 cycle/row at N>=256). PSUM accumulates fp32; output is fp32.
"""

import numpy as np
import ml_dtypes

import concourse.bass as bass
import concourse.bacc as bacc
import concourse.mybir as mybir
import concourse.tile as tile
from concourse.bass_utils import run_bass_kernel_spmd

# Problem constants (hardcoded per the harness contract).
N_CORES = 8
B = 16
IMGS_PER_CORE = B // N_CORES  # 2
CIN = 16      # conv input channels after gather
COUT = 32     # filters
H = W = 256
SLOTS = 8     # strips per image
HS = H // SLOTS  # 32 output rows per strip
ROWS = HS + 2    # strip buffer rows incl. top/bottom halo
WP = W + 2       # padded row width
HALF = 16        # output rows per store chunk
TAPS = [(dh, dw) for dh in range(3) for dw in range(3)]

# Matmul operand mode: both operands must have matching width (compiler
# rejects mixed 32-bit / 16-bit). "f32r" = fp32 storage, float32r matmul
# (full speed at N>=256, near-fp32 accuracy). "bf16" = both bf16.
MODE = "bf16"


def _common_pairs(ms_in, ns_in, ms_x, ns_x):
    ms_in = np.asarray(ms_in)
    ns_in = np.asarray(ns_in)
    ms_x = np.asarray(ms_x)
    ns_x = np.asarray(ns_x)
    f_ids, x_ids = [], []
    for i_in in range(ms_in.shape[0]):
        hits = np.nonzero((ms_x == ms_in[i_in]) & (ns_x == ns_in[i_in]))[0]
        for i_x in hits:
            f_ids.append(i_in)
            x_ids.append(int(i_x))
    return np.asarray(f_ids), np.asarray(x_ids)


def build_program(n_img=IMGS_PER_CORE, mode=MODE):
    """Build the per-core Bass program. Returns compiled Bacc."""
    n_strips = SLOTS
    hs = HS
    rows = hs + 2
    h_img = n_strips * hs

    f32 = mybir.dt.float32
    w_dt = mybir.dt.float32r if mode == "f32r" else mybir.dt.bfloat16
    x_sb_dt = w_dt  # matmul operands must have matching width/dtype class

    nc = bacc.Bacc("TRN2", target_bir_lowering=False, debug=False)
    x_in = nc.dram_tensor("x", [n_img, CIN, h_img, W], f32, kind="ExternalInput")
    w_in = nc.dram_tensor("w", [128, 9, 128], f32, kind="ExternalInput")
    y_out = nc.dram_tensor("y", [n_img, COUT, h_img, W], f32, kind="ExternalOutput")

    # h_abs = 128*c + 32*gg + 16*m + r  ->  (c, gg, m, r)
    y_r = y_out[:].rearrange(
        "b co (c gg m r) w -> b c gg m co r w", c=2, gg=4, m=hs // HALF, r=HALF
    )

    with tile.TileContext(nc) as tc:
        with (
            tc.tile_pool(name="persist", bufs=1) as persist,
            tc.tile_pool(name="op0", bufs=2) as op0,
            tc.tile_pool(name="op1", bufs=2) as op1,
            tc.tile_pool(name="ps0", bufs=4, space="PSUM") as ps0,
            tc.tile_pool(name="ps1", bufs=4, space="PSUM") as ps1,
        ):
            out_pools = [op0, op1]
            ps_pools = [ps0, ps1]

            wt = persist.tile([128, 9, 128], w_dt, name="wt")
            nc.gpsimd.dma_start(out=wt[:], in_=w_in[:])  # casts f32 -> f32r/bf16

            xbufs = []
            for p in range(n_img):
                xb = persist.tile([128, rows * WP], x_sb_dt, name=f"xb{p}")
                xbufs.append(xb)
                xv = xb[:].rearrange("q (r c) -> q r c", c=WP)
                # zero pads: left/right columns, top halo of slot 0,
                # bottom halo of slot 7. memset can't target f32r directly
                # (ISA check), so write zero bits through a uint32 view.
                if mode == "f32r":
                    zv = lambda ap: (ap.bitcast(mybir.dt.uint32), 0)
                else:
                    zv = lambda ap: (ap, 0.0)
                nc.vector.memset(*zv(xv[:, :, 0:1]))
                nc.vector.memset(*zv(xv[:, :, WP - 1 : WP]))
                # 32-aligned partition spans (DVE base-partition constraint);
                # the non-halo half of each span is overwritten by the loads.
                nc.vector.memset(*zv(xv[0:32, 0:1, :]))
                nc.vector.memset(*zv(xv[96:128, rows - 1 : rows, :]))

            # Input loads for all phases up front (prefetch).
            for p in range(n_img):
                xv = xbufs[p][:].rearrange("q (r c) -> q r c", c=WP)
                for g in range(n_strips):
                    r0 = 1 if g == 0 else 0
                    r1 = rows - 1 if g == n_strips - 1 else rows
                    i0 = g * hs - 1 + r0  # first image row
                    dst = xv[CIN * g : CIN * (g + 1), r0:r1, 1 : W + 1]
                    src = x_in[p, :, i0 : i0 + (r1 - r0), :]
                    nc.gpsimd.dma_start(out=dst, in_=src)  # casts f32 -> f32r/bf16

            for p in range(n_img):
                xr = xbufs[p][:]
                outt = [None, None]
                for h in range(hs):
                    m, r = divmod(h, HALF)
                    for c in (0, 1):
                        ps = ps_pools[c].tile([128, W], f32, name=f"acc{c}")
                        for t, (dh, dw) in enumerate(TAPS):
                            col0 = (h + dh) * WP + dw
                            nc.tensor.matmul(
                                ps[:],
                                wt[64 * c : 64 * (c + 1), t, :],
                                xr[64 * c : 64 * (c + 1), col0 : col0 + W],
                                start=(t == 0),
                                stop=(t == 8),
                            )
                        if r == 0:
                            outt[c] = out_pools[c].tile(
                                [128, HALF * W], f32, name=f"ot{c}"
                            )
                        nc.vector.tensor_copy(outt[c][:, r * W : (r + 1) * W], ps[:])
                        if r == HALF - 1:
                            nc.sync.dma_start(out=y_r[p, c, :, m], in_=outt[c][:])

    nc.compile()
    return nc


_NC_CACHE = {}


def _get_program(mode=MODE):
    if mode not in _NC_CACHE:
        _NC_CACHE[mode] = build_program(mode=mode)
    return _NC_CACHE[mode]


def _host_prep(inputs):
    x = np.asarray(inputs["x_data"], dtype=np.float32)
    w = np.asarray(inputs["weights"], dtype=np.float32)
    f_ids, x_ids = _common_pairs(
        inputs["ms_in"], inputs["ns_in"], inputs["ms_x"], inputs["ns_x"]
    )
    assert len(f_ids) == CIN, f"expected {CIN} matched pairs, got {len(f_ids)}"
    xg = np.ascontiguousarray(x[:, x_ids])          # [B, 16, H, W]
    wg = w[:, f_ids]                                # [COUT, 16, 3, 3]

    # Stationary per tap: block-diag of 4 copies of W_t.T [16, 32] -> [64, 128],
    # replicated on partitions 64-127 for the second chain.
    w_host = np.zeros((128, 9, 128), dtype=np.float32)
    for t, (dh, dw) in enumerate(TAPS):
        lhsT = wg[:, :, dh, dw].T  # [16 ci, 32 co]
        for g in range(4):
            blk = w_host[16 * g : 16 * (g + 1), t, 32 * g : 32 * (g + 1)]
            blk[:] = lhsT
    w_host[64:128] = w_host[0:64]
    return xg, w_host


def _run(inputs, trace=False):
    xg, w_host = _host_prep(inputs)
    nc = _get_program()
    in_maps = [
        {"x": xg[IMGS_PER_CORE * k : IMGS_PER_CORE * (k + 1)], "w": w_host}
        for k in range(N_CORES)
    ]
    res = run_bass_kernel_spmd(nc, in_maps, list(range(N_CORES)), trace=trace)
    out = np.concatenate([r["y"] for r in res.results], axis=0)
    return out, res


def kernel(**inputs):
    out, _ = _run(inputs, trace=False)
    return out


# revision 18
# speedup vs baseline: 1.9024x; 1.6734x over previous
"""Trainium2 Bass kernel for nn_DecentLayer (gnn_message_passing).

The reference gathers 16 of 24 input channels via static position matching,
then runs a 3x3 same-padded conv: [B=16, 16, 256, 256] x [32, 16, 3, 3]
-> [B, 32, 256, 256].

Strategy (v2):
  * Data-parallel over batch: 8 cores x 2 images ("phases").
  * Host pre-assembles the SBUF-ready input: per image, 8 horizontal strips
    of 32 output rows in a zero-padded row-major layout (258 cols x 34 rows
    incl. halos), pre-cast to bf16. Each strip occupies 16 partitions for
    the unshifted copy plus 16 partitions holding the same rows shifted by
    one column ("copy1"). Partition p = 32*gg + 16*cp + ch; strip slot
    = 2*gg + sg with sg indexed along the free dim. One contiguous DMA per
    phase -- full-rate descriptors, no on-chip padding or casts.
  * Conv = shifted matmuls accumulating in PSUM. K=128 block-diagonal
    stationary batches 4 strips x (16 ch x 2 copies); M=128 = 4 strips x 32
    filters. The shifted copy turns two horizontal taps into ONE matmul:
    per output row, 3 pair-matmuls (dw=0,1) + 3 single-matmuls (dw=2)
    instead of 9. All tap shifts are SBUF address offsets.
  * PSUM: one output row (N=256) per bank, 8 banks rotating; DVE evacuates
    to an output stage; SWDGE DMA (all 16 engines) stores to HBM.
"""

import numpy as np
import ml_dtypes

import concourse.bass as bass
import concourse.bacc as bacc
import concourse.mybir as mybir
import concourse.tile as tile
from concourse.bass_utils import run_bass_kernel_spmd

# Problem constants (hardcoded per the harness contract).
N_CORES = 8
B = 16
IMGS_PER_CORE = B // N_CORES  # 2
CIN = 16      # conv input channels after gather
COUT = 32     # filters
H = W = 256
SLOTS = 8     # strips per image
HS = H // SLOTS   # 32 output rows per strip
ROWS = HS + 2     # strip rows incl. halo
WP = W + 2        # padded row width
SSTRIDE = ROWS * WP  # 8772 elems per strip per partition
HALF = 16         # output rows per store chunk
N_TAPMM = 6       # matmuls per output row: 3 pairs + 3 singles

MODE = "bf16"  # "bf16" or "f32r" (fp32 storage streamed as float32r)


def _common_pairs(ms_in, ns_in, ms_x, ns_x):
    ms_in = np.asarray(ms_in)
    ns_in = np.asarray(ns_in)
    ms_x = np.asarray(ms_x)
    ns_x = np.asarray(ns_x)
    f_ids, x_ids = [], []
    for i_in in range(ms_in.shape[0]):
        hits = np.nonzero((ms_x == ms_in[i_in]) & (ns_x == ns_in[i_in]))[0]
        for i_x in hits:
            f_ids.append(i_in)
            x_ids.append(int(i_x))
    return np.asarray(f_ids), np.asarray(x_ids)


def build_program(n_img=IMGS_PER_CORE, mode=MODE):
    """Build the per-core Bass program. Returns compiled Bacc."""
    f32 = mybir.dt.float32
    if mode == "f32r":
        sb_dt, dram_dt = mybir.dt.float32r, f32  # DMA cast rounds to f32r
    else:
        sb_dt, dram_dt = mybir.dt.bfloat16, mybir.dt.bfloat16

    nc = bacc.Bacc("TRN2", target_bir_lowering=False, debug=False)
    x_in = nc.dram_tensor("x", [n_img, 128, 2 * SSTRIDE], dram_dt,
                          kind="ExternalInput")
    w_in = nc.dram_tensor("w", [128, N_TAPMM, 128], dram_dt,
                          kind="ExternalInput")
    y_out = nc.dram_tensor("y", [n_img, COUT, H, W], f32, kind="ExternalOutput")

    # h_abs = 64*gg + 32*sg + 16*m + r   (strip slot = 2*gg + sg)
    y_r = y_out[:].rearrange(
        "b co (gg sg m r) w -> b sg m gg co r w", gg=4, sg=2, m=HS // HALF, r=HALF
    )

    with tile.TileContext(nc) as tc:
        with (
            tc.tile_pool(name="persist", bufs=1) as persist,
            tc.tile_pool(name="op", bufs=3) as op,
            tc.tile_pool(name="ps", bufs=8, space="PSUM") as psp,
        ):
            wt = persist.tile([128, N_TAPMM, 128], sb_dt, name="wt")
            nc.gpsimd.dma_start(out=wt[:], in_=w_in[:])

            xbufs = []
            for p in range(n_img):
                xb = persist.tile([128, 2 * SSTRIDE], sb_dt, name=f"xb{p}")
                xbufs.append(xb)
                nc.gpsimd.dma_start(out=xb[:], in_=x_in[p])

            for p in range(n_img):
                xr = xbufs[p][:]
                for sg in range(2):
                    outt = None
                    for h in range(HS):
                        m, r = divmod(h, HALF)
                        ps = psp.tile([128, W], f32, name="acc")
                        for t in range(N_TAPMM):
                            dh, dw0 = t % 3, (0 if t < 3 else 2)
                            o = sg * SSTRIDE + (h + dh) * WP + dw0
                            nc.tensor.matmul(
                                ps[:],
                                wt[:, t, :],
                                xr[:, o : o + W],
                                start=(t == 0),
                                stop=(t == N_TAPMM - 1),
                            )
                        if r == 0:
                            outt = op.tile([128, HALF * W], f32, name="ot")
                        nc.vector.tensor_copy(outt[:, r * W : (r + 1) * W], ps[:])
                        if r == HALF - 1:
                            nc.gpsimd.dma_start(out=y_r[p, sg, m], in_=outt[:])

    nc.compile()
    return nc


_NC_CACHE = {}


def _get_program(mode=MODE):
    if mode not in _NC_CACHE:
        _NC_CACHE[mode] = build_program(mode=mode)
    return _NC_CACHE[mode]


def _host_prep(inputs):
    x = np.asarray(inputs["x_data"], dtype=np.float32)
    w = np.asarray(inputs["weights"], dtype=np.float32)
    f_ids, x_ids = _common_pairs(
        inputs["ms_in"], inputs["ns_in"], inputs["ms_x"], inputs["ns_x"]
    )
    assert len(f_ids) == CIN, f"expected {CIN} matched pairs, got {len(f_ids)}"
    xg = x[:, x_ids]                                 # [B, 16, H, W]
    wg = w[:, f_ids]                                 # [COUT, 16, 3, 3]

    np_dt = ml_dtypes.bfloat16 if MODE == "bf16" else np.float32
    xc = xg.astype(np_dt)

    # SBUF-ready layout: [B, 128, 2, ROWS, WP]; partition = 32*gg + 16*cp + ch,
    # strip slot = 2*gg + sg; copy cp=1 holds the same rows shifted one column
    # left (value at col c = padded col c+1) so one matmul covers taps
    # (dh, dw) and (dh, dw+1).
    host = np.zeros((B, 128, 2, ROWS, WP), dtype=np_dt)
    for slot in range(SLOTS):
        gg, sg = divmod(slot, 2)
        r_lo = max(0, HS * slot - 1)
        r_hi = min(H, HS * slot + HS + 1)
        dst_r0 = r_lo - (HS * slot - 1)
        n = r_hi - r_lo
        rows = xc[:, :, r_lo:r_hi, :]
        p0 = 32 * gg
        host[:, p0 : p0 + 16, sg, dst_r0 : dst_r0 + n, 1 : W + 1] = rows
        host[:, p0 + 16 : p0 + 32, sg, dst_r0 : dst_r0 + n, 0:W] = rows
    host = host.reshape(B, 128, 2 * SSTRIDE)

    # Stationaries [128, 6, 128]: t in 0..2 = pair (W[dh,0] | W[dh,1]),
    # t in 3..5 = single (W[dh,2] | 0). Block-diagonal over 4 strips.
    w_host = np.zeros((128, N_TAPMM, 128), dtype=np.float32)
    for dh in range(3):
        for gg in range(4):
            q = 32 * gg
            w_host[q : q + 16, dh, q : q + 32] = wg[:, :, dh, 0].T
            w_host[q + 16 : q + 32, dh, q : q + 32] = wg[:, :, dh, 1].T
            w_host[q : q + 16, 3 + dh, q : q + 32] = wg[:, :, dh, 2].T
    w_host = w_host.astype(np_dt)
    return host, w_host


def _run(inputs, trace=False):
    xh, w_host = _host_prep(inputs)
    nc = _get_program()
    in_maps = [
        {"x": xh[IMGS_PER_CORE * k : IMGS_PER_CORE * (k + 1)], "w": w_host}
        for k in range(N_CORES)
    ]
    res = run_bass_kernel_spmd(nc, in_maps, list(range(N_CORES)), trace=trace)
    out = np.concatenate([r["y"] for r in res.results], axis=0)
    return out, res


def kernel(**inputs):
    out, _ = _run(inputs, trace=False)
    return out
